# Initial kernel scaffold
#
"""Multi-head attention + residual + batchnorm on 8 trn2 NeuronCores.

Sharding: core c handles batch b = c % 4 and head-group g = c // 4
(4 heads = 512 feature dims per group). All device compute happens in
feature-major ("transposed") space so every matmul contracts over the
partition dim with zero on-chip transposes:

  QT[u,t] = (Wq_g/sqrt(D)) @ query[b].T      (bf16 in, f32 psum)
  KT[u,t] = Wk_g @ keys[b].T                 (bf16)
  V[t,u]  = keys[b] @ Wv_g.T                 (bf16 in, f32 psum, bf16 out)
  ST[k,q] = KT_h.T-contract QT_h             (bf16; scores transposed)
  PT      = exp(ST)            (ACT, PSUM->SBUF, bf16; scores bounded, no max)
  OT[u,q] = sum_k V[k,u]*PT[k,q]             (bf16)
  r[q]    = sum_k PT[k,q]  via ones-matmul   (bf16)
  o_res   = OT/r + query[b].T slice          (f32)
  batchnorm over (b,s): local sums + 4-core AllReduce, then affine.

BatchNorm feature stats are local to a head-group, reduced across the 4
cores sharing g. All x/weight inputs are cast to bf16 on host (halves
DMA + SBUF traffic; matmuls accumulate in f32 PSUM). Tail/overlap
optimizations, each verified against the neuron-profile trace:
- exp ACT table preloaded at t=0 (and sqrt warmed during the collective
  wait) so the ACT engine never reloads its function table mid-pipeline.
- The cross-core stats exchange is an AllGather + local sums instead of
  an AllReduce: the cc stack charges AllReduce ~1.9x the latency of a
  gather of the same (tiny) payload (~28us -> ~14us measured), and
  summing the four gathered blocks locally is rank-order-independent.
- Each head's post-collective work (gather readback on the hw DGE
  queues, stats, sqrt, affine, writeout) is deferred to the end of the
  schedule via tile_wait_until so no engine queue idles mid-pipeline on
  a collective; affine+writeout run as halves split across DVE/ACT and
  both DMA queues.
- The final iteration's rowsum tree sums into a scratch tile instead of
  in-place on PT, so it overlaps the AV matmuls still reading PT and
  the last stats chain starts ~3us earlier.
- gamma/beta loads ride the idle gpsimd queue so the first projection
  x-tiles are not queued behind them at startup.

Known floors (measured): ~7.3us engine preamble, ~14us collective
latency, ~8us finalization drains. A remote_dma_broadcast peer exchange
could replace the collective (~5us); XOR-relative routing to peers
tpb^{1,2,3} was verified correct on this topology, but the arrival
semaphore releases before data lands (increments appear send-side), so
it needs an instrumented probe before it can be trusted.
"""
import sys

sys.path.insert(0, "/opt/trn_rl_repo")

import numpy as np

import concourse.bass as bass
import concourse.tile as tile
from concourse import bacc, mybir
from concourse.bass_utils import run_bass_kernel_spmd

F32 = mybir.dt.float32
F32R = mybir.dt.float32r
BF16 = mybir.dt.bfloat16
AF = mybir.ActivationFunctionType
NPBF16 = mybir.dt.np(mybir.dt.bfloat16)

B, S, D, H = 4, 2048, 1024, 8
DH = D // H          # 128
HG = 4               # heads per group (per core)
GF = HG * DH         # 512 features per group
EPS = 1e-5
P = 128
DT = 8               # d-tiles (D / 128)
TC = 4               # token chunks of 512
TCW = 512
KT_N = 16            # k tiles of 128 per sequence
NTOK = B * S         # batchnorm population per feature


def _build():
    nc = bacc.Bacc(num_swdge_queues=1)
    qt = nc.declare_dram_parameter("qt", [TC, DT, P, TCW], BF16, isOutput=False)
    kt = nc.declare_dram_parameter("kt", [TC, DT, P, TCW], BF16, isOutput=False)
    wq = nc.declare_dram_parameter("wq", [DT, P, GF], BF16, isOutput=False)
    wk = nc.declare_dram_parameter("wk", [DT, P, GF], BF16, isOutput=False)
    wv = nc.declare_dram_parameter("wv", [DT, P, GF], BF16, isOutput=False)
    qres = nc.declare_dram_parameter("qres", [HG, TC, P, TCW], F32, isOutput=False)
    gamma = nc.declare_dram_parameter("gamma", [P, HG], F32, isOutput=False)
    beta = nc.declare_dram_parameter("beta", [P, HG], F32, isOutput=False)
    out = nc.declare_dram_parameter("out", [P, HG, S], F32, isOutput=True)


    with tile.TileContext(nc) as tc:
        with (
            tc.tile_pool(name="persist", bufs=1) as persist,
            tc.tile_pool(name="dram", bufs=1, space="DRAM") as dram,
        ):
            # ---- persistent SBUF ----
            QT = persist.tile([P, HG, S], BF16)          # (dh, h, q) 16KB/p
            KTb = persist.tile([P, HG, S], BF16)         # (dh, h, k) 16KB/p
            V = persist.tile([P, KT_N, GF], BF16)        # (t128, kt, u) 16KB/p
            o_res = persist.tile([P, HG, S], F32)        # 32KB/p
            gam = persist.tile([P, HG], F32)
            bet = persist.tile([P, HG], F32)
            ones_f = persist.tile([P, P], F32)
            ones_b = persist.tile([P, P], BF16)
            BN_DIM = nc.vector.BN_STATS_DIM
            cc_in = [
                dram.tile([P, TC, BN_DIM], F32, name=f"cc_in{h}")
                for h in range(HG)
            ]
            cc_out = [
                dram.tile([4, P, TC, BN_DIM], F32, name=f"cc_out{h}")
                for h in range(HG)
            ]

            nc.gpsimd.dma_start(gam[:], gamma[:])
            nc.gpsimd.dma_start(bet[:], beta[:])
            eps_t = persist.tile([P, 1], F32)
            nc.vector.memset(eps_t[:], float(EPS))
            nc.vector.memset(ones_f[:], 1.0)
            nc.vector.tensor_copy(ones_b[:], ones_f[:])
            rs8 = persist.tile([P, 8, TCW], BF16)
            warm = persist.tile([P, 1], F32)
            nc.scalar.activation(out=warm[:], in_=eps_t[:], func=AF.Exp)

            # ---- phase 1: projections (stream qt/kt, weights resident) ----
            with (
                tc.tile_pool(name="wpool", bufs=1) as wpool,
                tc.tile_pool(name="xstream", bufs=20) as xstream,
                tc.tile_pool(name="ppsum", bufs=4, space="PSUM") as ppsum,
                tc.tile_pool(name="ptmp", bufs=4) as ptmp,
            ):
                wq_s = [wpool.tile([P, GF], BF16, name=f"wq{d}") for d in range(DT)]
                wk_s = [wpool.tile([P, GF], BF16, name=f"wk{d}") for d in range(DT)]
                wv_s = [wpool.tile([P, GF], BF16, name=f"wv{d}") for d in range(DT)]
                wq_r, wk_r, wv_r = wq, wk, wv

                # Q^T
                for tc_i in range(TC):
                    xh = []
                    if tc_i == 0:
                        for d in range(DT):
                            weng = nc.scalar if d % 2 == 0 else nc.sync
                            weng.dma_start(wq_s[d][:], wq_r[d])
                            xeng = nc.sync if d % 2 == 0 else nc.scalar
                            t = xstream.tile([P, TCW], BF16, tag="x")
                            xeng.dma_start(t[:], qt[tc_i, d])
                            xh.append(t)
                    if tc_i == 1:
                        for d in range(DT):
                            eng = nc.scalar if d % 2 == 0 else nc.sync
                            eng.dma_start(wk_s[d][:], wk_r[d])
                    elif tc_i == 2:
                        for d in range(DT):
                            eng = nc.scalar if d % 2 == 0 else nc.sync
                            eng.dma_start(wv_s[d][:], wv_r[d])
                    if tc_i > 0:
                        for d in range(DT):
                            t = xstream.tile([P, TCW], BF16, tag="x")
                            eng = nc.sync if d % 2 == 0 else nc.scalar
                            eng.dma_start(t[:], qt[tc_i, d])
                            xh.append(t)
                    for h in range(HG):
                        ps = ppsum.tile([P, TCW], F32)
                        for d in range(DT):
                            nc.tensor.matmul(
                                ps[:],
                                wq_s[d][:, bass.ts(h, DH)],
                                xh[d][:],
                                start=(d == 0),
                                stop=(d == DT - 1),
                            )
                        nc.vector.tensor_copy(
                            QT[:, h, bass.ts(tc_i, TCW)], ps[:]
                        )
                # K^T and V
                for tc_i in range(TC):
                    xh = []
                    for d in range(DT):
                        t = xstream.tile([P, TCW], BF16, tag="x")
                        eng = nc.sync if d % 2 == 0 else nc.scalar
                        eng.dma_start(t[:], kt[tc_i, d])
                        xh.append(t)
                    for h in range(HG):
                        ps = ppsum.tile([P, TCW], F32)
                        for d in range(DT):
                            nc.tensor.matmul(
                                ps[:],
                                wk_s[d][:, bass.ts(h, DH)],
                                xh[d][:],
                                start=(d == 0),
                                stop=(d == DT - 1),
                            )
                        nc.vector.tensor_copy(
                            KTb[:, h, bass.ts(tc_i, TCW)], ps[:]
                        )
                    for sub in range(TCW // P):  # 4 t128 tiles in this chunk
                        kt_idx = tc_i * (TCW // P) + sub
                        psv = ppsum.tile([P, GF], F32)
                        for d in range(DT):
                            nc.tensor.matmul(
                                psv[:],
                                xh[d][:, bass.ts(sub, P)],
                                wv_s[d][:],
                                start=(d == 0),
                                stop=(d == DT - 1),
                            )
                        nc.vector.tensor_copy(V[:, kt_idx, :], psv[:])

            # ---- phase 2: attention + residual ----
            with (
                tc.tile_pool(name="pt_pool", bufs=2) as pt_pool,
                tc.tile_pool(name="qr_pool", bufs=3) as qr_pool,
                tc.tile_pool(name="spsum", bufs=2, space="PSUM") as spsum,
                tc.tile_pool(name="opsum", bufs=2, space="PSUM") as opsum,
                tc.tile_pool(name="rpsum", bufs=2, space="PSUM") as rpsum,
                tc.tile_pool(name="small", bufs=4) as small,
            ):
                for h in range(HG):
                    bstat = small.tile(
                        [P, TC, nc.vector.BN_STATS_DIM], F32, tag="bstat"
                    )
                    for q_i in range(TC):
                        PT = pt_pool.tile([P, KT_N, TCW], BF16, tag="pt")
                        for kp in range(KT_N // 2):
                            ps_s = spsum.tile([P, 2, TCW], F32, tag="s")
                            for j in range(2):
                                nc.tensor.matmul(
                                    ps_s[:, j, :],
                                    KTb[:, h, bass.ts(2 * kp + j, P)],
                                    QT[:, h, bass.ts(q_i, TCW)],
                                    start=True,
                                    stop=True,
                                )
                            nc.scalar.activation(
                                out=PT[:, 2 * kp : 2 * kp + 2, :],
                                in_=ps_s[:],
                                func=AF.Exp,
                            )
                        ps_o = opsum.tile([P, TCW], F32, tag="o")
                        for k in range(KT_N):
                            nc.tensor.matmul(
                                ps_o[:],
                                V[:, k, bass.ts(h, DH)],
                                PT[:, k, :],
                                start=(k == 0),
                                stop=(k == KT_N - 1),
                            )
                        # pairwise tree over the 16 k-tiles of PT; the
                        # final iteration sums into a scratch tile instead
                        # of in-place so the tree overlaps the AV matmuls
                        # still reading PT (shortens the last stats chain)
                        rpart = small.tile([P, TCW], BF16, tag="rpart")
                        if h == HG - 1 and q_i == TC - 1:
                            for j in range(8):
                                nc.vector.tensor_add(
                                    rs8[:, j, :],
                                    PT[:, 2 * j, :],
                                    PT[:, 2 * j + 1, :],
                                )
                            for j in range(4):
                                nc.vector.tensor_add(
                                    rs8[:, j, :], rs8[:, j, :], rs8[:, j + 4, :]
                                )
                            nc.vector.tensor_add(
                                rs8[:, 0, :], rs8[:, 0, :], rs8[:, 2, :]
                            )
                            nc.vector.tensor_add(
                                rpart[:], rs8[:, 0, :], rs8[:, 1, :]
                            )
                            nc.vector.tensor_add(
                                rpart[:], rpart[:], rs8[:, 3, :]
                            )
                        else:
                            for step in (1, 2, 4, 8):
                                for j in range(0, KT_N, 2 * step):
                                    if step < 8:
                                        nc.vector.tensor_add(
                                            PT[:, j, :],
                                            PT[:, j, :],
                                            PT[:, j + step, :],
                                        )
                            nc.vector.tensor_add(
                                rpart[:], PT[:, 0, :], PT[:, 8, :]
                            )
                        ps_r = rpsum.tile([P, TCW], F32, tag="r")
                        nc.tensor.matmul(
                            ps_r[:], ones_b[:], rpart[:], start=True, stop=True
                        )
                        rb = small.tile([P, TCW], F32, tag="rb")
                        nc.vector.reciprocal_approx_fast(out=rb[:], in_=ps_r[:])
                        qres_ch = qr_pool.tile([P, TCW], F32, tag="qres")
                        nc.sync.dma_start(qres_ch[:], qres[h, q_i])
                        dst = o_res[:, h, bass.ts(q_i, TCW)]
                        nc.vector.tensor_tensor(
                            dst, ps_o[:], rb[:], mybir.AluOpType.mult
                        )
                        nc.vector.tensor_add(dst, dst, qres_ch[:])
                        nc.vector.bn_stats(out=bstat[:, q_i, :], in_=dst)

                    # per-head stats exchange (overlaps later heads):
                    # ship the raw bn_stats blocks; bn_aggr combines the
                    # 16 gathered entries exactly, so no sum/sumsq
                    # conversion is needed on the critical chain
                    nc.gpsimd.dma_start(cc_in[h][:], bstat[:])
                    # AllGather + local sum: the cc stack charges
                    # AllReduce ~1.9x the latency of a gather of the same
                    # (tiny) payload; summing the four gathered blocks
                    # locally is order-independent so rank layout is moot
                    nc.gpsimd.collective_compute(
                        "AllGather",
                        mybir.AluOpType.bypass,
                        ins=[cc_in[h].opt()],
                        outs=[cc_out[h].opt()],
                        replica_groups=[[0, 1, 2, 3], [4, 5, 6, 7]],
                    )
                    # post-AllReduce work: pushed late in the schedule so
                    # no engine queue idles mid-pipeline waiting on the
                    # collective
                    with tc.tile_wait_until(0.9 + 0.02 * h):
                        gstall = small.tile(
                            [P, 4 * TC, BN_DIM], F32, tag="gstall"
                        )
                        for r in range(4):
                            eng = nc.sync if r % 2 == 0 else nc.scalar
                            eng.dma_start(
                                gstall[:, bass.ts(r, TC), :], cc_out[h][r]
                            )
                        mv = small.tile([P, 2], F32, tag="mv")
                        nc.vector.bn_aggr(out=mv[:], in_=gstall[:])
                        mean = mv[:, 0:1]
                        std = small.tile([P, 1], F32, tag="std")
                        nc.scalar.activation(
                            out=std[:], in_=mv[:, 1:2], func=AF.Sqrt,
                            bias=eps_t[:],
                        )
                        rstd = small.tile([P, 1], F32, tag="rstd")
                        nc.vector.reciprocal(out=rstd[:], in_=std[:])
                        scale = small.tile([P, 1], F32, tag="scale")
                        shift = small.tile([P, 1], F32, tag="shift")
                        nc.vector.tensor_mul(scale[:], gam[:, h : h + 1], rstd[:])
                        nc.vector.tensor_mul(shift[:], mean, scale[:])
                        nc.vector.tensor_sub(
                            shift[:], bet[:, h : h + 1], shift[:]
                        )

                        for half in range(2):
                            sl = bass.ts(half, S // 2)
                            if half == 0:
                                nc.vector.tensor_scalar(
                                    o_res[:, h, sl],
                                    o_res[:, h, sl],
                                    scale[:],
                                    shift[:],
                                    mybir.AluOpType.mult,
                                    mybir.AluOpType.add,
                                )
                            else:
                                nc.scalar.activation(
                                    out=o_res[:, h, sl],
                                    in_=o_res[:, h, sl],
                                    func=AF.Identity,
                                    bias=shift[:],
                                    scale=scale[:],
                                )
                            weng = nc.sync if half == 0 else nc.scalar
                            weng.dma_start(out[:, h, sl], o_res[:, h, sl])

                with tc.tile_wait_until(0.85):
                    nc.scalar.activation(
                        out=warm[:], in_=eps_t[:], func=AF.Sqrt
                    )

    nc.finalize()
    return nc


_NC = None


def _get_nc():
    global _NC
    if _NC is None:
        _NC = _build()
    return _NC


def _make_in_maps(query, keys, Wq, Wk, Wv, gamma, beta):
    query = np.asarray(query, dtype=np.float32)
    keys = np.asarray(keys, dtype=np.float32)
    Wq = np.asarray(Wq, dtype=np.float32)
    Wk = np.asarray(Wk, dtype=np.float32)
    Wv = np.asarray(Wv, dtype=np.float32)
    gamma = np.asarray(gamma, dtype=np.float32)
    beta = np.asarray(beta, dtype=np.float32)

    scale = 1.0 / np.sqrt(np.float32(D))
    in_maps = []
    for c in range(8):
        b, g = c % B, c // B
        rows = slice(GF * g, GF * (g + 1))
        qt = np.ascontiguousarray(query[b].T)              # (D, S)
        kt = np.ascontiguousarray(keys[b].T)
        def tile4(x):  # (D, S) -> (TC, DT, 128, TCW) contiguous bf16
            return np.ascontiguousarray(
                x.reshape(DT, P, TC, TCW).transpose(2, 0, 1, 3).astype(NPBF16)
            )
        def tilew(w):  # (D, GF) -> (DT, 128, GF) contiguous bf16
            return np.ascontiguousarray(w.reshape(DT, P, GF).astype(NPBF16))
        qres_f = qt[rows]                                   # (GF, S)
        qres4 = np.ascontiguousarray(
            qres_f.reshape(HG, P, TC, TCW).transpose(0, 2, 1, 3)
        )
        in_maps.append(
            {
                "qt": tile4(qt),
                "kt": tile4(kt),
                "wq": tilew(np.ascontiguousarray(Wq[rows].T * scale)),
                "wk": tilew(np.ascontiguousarray(Wk[rows].T)),
                "wv": tilew(np.ascontiguousarray(Wv[rows].T)),
                "qres": qres4,
                "gamma": np.ascontiguousarray(
                    gamma[rows].reshape(HG, P).T
                ),
                "beta": np.ascontiguousarray(beta[rows].reshape(HG, P).T),
            }
        )
    return in_maps


def _run(in_maps, trace=False, **kw):
    nc = _get_nc()
    return run_bass_kernel_spmd(
        nc, in_maps, core_ids=list(range(8)), trace=trace, **kw
    )


def kernel(query, keys, Wq, Wk, Wv, gamma, beta):
    in_maps = _make_in_maps(query, keys, Wq, Wk, Wv, gamma, beta)
    res = _run(in_maps)
    output = np.empty((B, S, D), dtype=np.float32)
    for c in range(8):
        b, g = c % B, c // B
        oc = res.results[c]["out"]                   # (128, 4, 2048)
        block = oc.transpose(2, 1, 0).reshape(S, GF)  # (S, GF): [t, h*128+p]
        output[b, :, GF * g : GF * (g + 1)] = block
    return output



# revision 10
# speedup vs baseline: 1.0673x; 1.0673x over previous
"""Multi-head attention + residual + batchnorm on 8 trn2 NeuronCores.

Sharding: core c owns head h = c for ALL 4 batches. Head h covers output
features [h*128, (h+1)*128), so batchnorm statistics over (batch, seq)
are fully local to the core: no cross-core collective at all.

All device compute is feature-major so every matmul contracts over the
partition dim with zero on-chip transposes:

  QT[u,t] = Wq_h @ query[b].T     fp8 DoubleRow (K=256/pass), descaled
  KT[u,t] = Wk_h @ keys[b].T      fp8 DoubleRow from on-chip-cast keys
  V[t,u]  = keys[b] @ Wv_h.T      bf16 (N=128 matmuls, FWL), stored fp8
  ST[k,q] = KT.T-contract QT      bf16 (K=128: DoubleRow not applicable)
  PT      = exp(ST)               ACT, PSUM->SBUF, fp8 (scores bounded)
  OT[u,q] = sum_k V[k,u]*PT[k,q]  fp8 DoubleRow
  r[q]    = sum_k PT[k,q]         fp8 DoubleRow ones-matmuls (f32 acc)
  o_res   = OT/r + query[b].T     f32 residual
  batchnorm over (b,s): bn_stats per chunk, local bn_aggr, affine.

fp8 scaling: weights are scaled x32 on host (unit std, fits e4m3);
the 1/32 score scale plus the x32x32 weight descale is folded into the
QT/KT PSUM copy-out factor sq = sk = 1/sqrt(32768).

DMA strategy (per-queue throughput is descriptor-rate-bound, ~23
descriptors/us): keys ship once as bf16 (the K-projection's fp8 copy
is cast on-chip by the DVE), and the host layouts put [P] outermost so
every DMA moves 4-16KB per partition in one descriptor. Streams are
spread over the gpsimd (keys), sync (query-fp8), and scalar (residual
+ weights) queues.

The tensor-engine emission interleaves, between the score matmuls of
chunk N, the AV+rowsum drain of chunk N-1 plus projection work for
upcoming batches, so the PE never stalls on the ACT engine (exp is the
per-chunk ACT straggler) freeing score PSUM banks, and the HAM clock
gate stays at 8/8.
"""
import sys

sys.path.insert(0, "/opt/trn_rl_repo")

from collections import deque

import numpy as np

import concourse.bass as bass
import concourse.tile as tile
from concourse import bacc, mybir
from concourse.bass_utils import run_bass_kernel_spmd

F32 = mybir.dt.float32
BF16 = mybir.dt.bfloat16
FP8 = mybir.dt.float8e4
AF = mybir.ActivationFunctionType
PM_DR = mybir.MatmulPerfMode.DoubleRow
ALU = mybir.AluOpType
NPBF16 = mybir.dt.np(BF16)
NPFP8 = mybir.dt.np(FP8)

B, S, D, H = 4, 2048, 1024, 8
DH = 128
P = 128
TC = 4                # 512-token chunks per sequence
TCW = 512
DT = 8                # 128-wide d-tiles in D
DP = 4                # d-tile pairs (DoubleRow K=256)
KT_N = 16             # 128-wide k-tiles per sequence
EPS = 1e-5
WSCALE = 32.0
SQK = 1.0 / np.sqrt(32768.0)   # QT/KT copy-out descale; sq*sk*D = 1/32


def _build():
    nc = bacc.Bacc(num_swdge_queues=1)
    qt8 = nc.declare_dram_parameter(
        "qt8", [B, P, TC, DP, 2, TCW], FP8, isOutput=False)
    kt16 = nc.declare_dram_parameter(
        "kt16", [B, P, TC, DT, TCW], BF16, isOutput=False)
    wq8 = nc.declare_dram_parameter("wq8", [P, DP, 2, DH], FP8, isOutput=False)
    wk8 = nc.declare_dram_parameter("wk8", [P, DP, 2, DH], FP8, isOutput=False)
    wv16 = nc.declare_dram_parameter("wv16", [P, DT, DH], BF16, isOutput=False)
    qres = nc.declare_dram_parameter("qres", [B, P, S], F32, isOutput=False)
    gamma = nc.declare_dram_parameter("gamma", [P, 1], F32, isOutput=False)
    beta = nc.declare_dram_parameter("beta", [P, 1], F32, isOutput=False)
    out = nc.declare_dram_parameter("out", [P, B, S], F32, isOutput=True)

    with tile.TileContext(nc) as tc:
        with (
            tc.tile_pool(name="persist", bufs=1) as persist,
            tc.tile_pool(name="xq8", bufs=2) as xq8p,       # per-batch tiles
            tc.tile_pool(name="xk16", bufs=6) as xk16p,     # per-tc tiles
            tc.tile_pool(name="kf8", bufs=6) as kf8p,       # cast scratch
            tc.tile_pool(name="pt", bufs=2) as ptp,
            tc.tile_pool(name="rb", bufs=2) as rbp,
            tc.tile_pool(name="otmp", bufs=2) as otmpp,
            tc.tile_pool(name="ppsum", bufs=2, space="PSUM") as ppsum,
            tc.tile_pool(name="spsum", bufs=2, space="PSUM") as spsum,
            tc.tile_pool(name="opsum", bufs=2, space="PSUM") as opsum,
        ):
            # ---- persistent SBUF ----
            QT = persist.tile([P, B, S], BF16)            # (dh, b, q) 16KB/p
            KT = persist.tile([P, B, KT_N, P], BF16)      # (dh, b, kt, k) 16KB/p
            V8 = persist.tile([P, B, KT_N, DH], FP8)      # (t128, b, kt, u) 8KB/p
            o_res = persist.tile([P, B, S], F32)          # 32KB/p
            bstat = persist.tile([P, B * TC, nc.vector.BN_STATS_DIM], F32)
            wq_s = persist.tile([P, DP, 2, DH], FP8)
            wk_s = persist.tile([P, DP, 2, DH], FP8)
            wv_s = persist.tile([P, DT, DH], BF16)
            gam = persist.tile([P, 1], F32)
            bet = persist.tile([P, 1], F32)
            ones_b = persist.tile([P, P], BF16)
            ones8 = persist.tile([P, 2, P], FP8)
            eps_t = persist.tile([P, 1], F32)
            warm = persist.tile([P, 1], F32)
            mv = persist.tile([P, 2], F32)
            stdt = persist.tile([P, 1], F32)
            rstd = persist.tile([P, 1], F32)
            scl = persist.tile([P, 1], F32)
            shf = persist.tile([P, 1], F32)

            # ---- preamble ----
            nc.vector.memset(eps_t[:], float(EPS))
            nc.vector.memset(ones_b[:], 1.0)
            nc.vector.memset(ones8[:], 1.0)

            xtiles = {}   # streamed tiles keyed by (kind, b[, tc])

            # batch-0 streams land first; queue order is tuned so the
            # earliest-needed tiles complete earliest: sync starts with
            # the first two query chunks, gpsimd/scalar carry keys
            qeng = [nc.gpsimd, nc.sync, nc.scalar, nc.gpsimd]
            t0q = xq8p.tile([P, TC, DP, 2, TCW], FP8, tag="xq", name="t0q")
            nc.sync.dma_start(t0q[:, 0], qt8[0, :, 0])
            nc.sync.dma_start(t0q[:, 1], qt8[0, :, 1])
            xtiles[("q8", 0)] = t0q
            k0t = []
            for tci in range(TC):
                t = xk16p.tile([P, DT, TCW], BF16, tag="xk16", name="t")
                xtiles[("k16", 0, tci)] = t
                k0t.append(t)
            nc.gpsimd.dma_start(k0t[0][:], kt16[0, :, 0])
            nc.scalar.dma_start(wk_s[:], wk8[:])
            nc.scalar.dma_start(wq_s[:], wq8[:])
            nc.sync.dma_start(k0t[1][:], kt16[0, :, 1])
            nc.scalar.dma_start(k0t[2][:], kt16[0, :, 2])
            nc.gpsimd.dma_start(k0t[3][:], kt16[0, :, 3])
            nc.sync.dma_start(t0q[:, 2], qt8[0, :, 2])
            nc.sync.dma_start(t0q[:, 3], qt8[0, :, 3])
            nc.scalar.dma_start(wv_s[:], wv16[:])
            nc.gpsimd.dma_start(o_res[:, 0, :], qres[0])
            nc.scalar.dma_start(gam[:], gamma[:])
            nc.scalar.dma_start(bet[:], beta[:])
            nc.scalar.activation(out=warm[:], in_=eps_t[:], func=AF.Exp)

            # PE warmup: pull the HAM clock gate to 8/8 before real work
            wps = ppsum.tile([P, TCW], F32, tag="pp", name="wps")
            for _ in range(32):
                nc.tensor.matmul(
                    wps[:, 0:P], ones_b[:], ones_b[:],
                    start=True, stop=True, skip_group_check=True,
                )

            def dma_batch(b):
                for tci in range(TC):
                    t = xk16p.tile([P, DT, TCW], BF16, tag="xk16")
                    qeng[(b + tci) % 3].dma_start(t[:], kt16[b, :, tci])
                    xtiles[("k16", b, tci)] = t
                t = xq8p.tile([P, TC, DP, 2, TCW], FP8, tag="xq")
                nc.sync.dma_start(t[:], qt8[b])
                xtiles[("q8", b)] = t
                nc.scalar.dma_start(o_res[:, b, :], qres[b])

            # ---- tensor-work units (each ~0.3-0.7us of PE time) ----
            def kproj_units(b):
                units = []
                for tci in range(TC):
                    ps = [None]

                    def u1(b=b, tci=tci, ps=ps):
                        ps[0] = ppsum.tile([P, TC, P], F32, tag="pp", name="pk")
                        xt = xtiles[("k16", b, tci)]
                        for dp in range(2):
                            kf = kf8p.tile([P, 2, TCW], FP8, tag="kf", name="kf")
                            nc.vector.tensor_copy(
                                kf[:], xt[:, bass.ts(dp, 2), :]
                            )
                            nc.tensor.matmul(
                                ps[0][:], wk_s[:, dp], kf[:],
                                start=(dp == 0), stop=False,
                                perf_mode=PM_DR, skip_group_check=True,
                            )

                    def u2(b=b, tci=tci, ps=ps):
                        xt = xtiles[("k16", b, tci)]
                        for dp in range(2, DP):
                            kf = kf8p.tile([P, 2, TCW], FP8, tag="kf", name="kf")
                            nc.vector.tensor_copy(
                                kf[:], xt[:, bass.ts(dp, 2), :]
                            )
                            nc.tensor.matmul(
                                ps[0][:], wk_s[:, dp], kf[:],
                                start=False, stop=(dp == DP - 1),
                                perf_mode=PM_DR, skip_group_check=True,
                            )
                        nc.vector.tensor_scalar(
                            KT[:, b, bass.ts(tci, TC), :], ps[0][:],
                            float(SQK), None, ALU.mult,
                        )

                    units += [u1, u2]
                return units

            def qproj_units(b, tci):
                ps = [None]

                def u1(b=b, tci=tci, ps=ps):
                    ps[0] = ppsum.tile([P, TCW], F32, tag="pp", name="pq")
                    xt = xtiles[("q8", b)]
                    for dp in range(2):
                        nc.tensor.matmul(
                            ps[0][:], wq_s[:, dp], xt[:, tci, dp],
                            start=(dp == 0), stop=False,
                            perf_mode=PM_DR, skip_group_check=True,
                        )

                def u2(b=b, tci=tci, ps=ps):
                    xt = xtiles[("q8", b)]
                    for dp in range(2, DP):
                        nc.tensor.matmul(
                            ps[0][:], wq_s[:, dp], xt[:, tci, dp],
                            start=False, stop=(dp == DP - 1),
                            perf_mode=PM_DR, skip_group_check=True,
                        )
                    nc.vector.tensor_scalar(
                        QT[:, b, bass.ts(tci, TCW)], ps[0][:],
                        float(SQK), None, ALU.mult,
                    )

                return [u1, u2]

            def vproj_units(b):
                units = []
                for tci in range(TC):
                    ps = [None]
                    for sub in range(4):

                        def u(b=b, tci=tci, sub=sub, ps=ps):
                            if sub == 0:
                                ps[0] = ppsum.tile([P, 4, DH], F32, tag="pp",
                                                   name="pv")
                            xt = xtiles[("k16", b, tci)]
                            for d in range(DT):
                                nc.tensor.matmul(
                                    ps[0][:, sub, :],
                                    xt[:, d, bass.ts(sub, P)],
                                    wv_s[:, d, :],
                                    start=(d == 0), stop=(d == DT - 1),
                                    skip_group_check=True,
                                )
                            if sub == 3:
                                nc.vector.tensor_copy(
                                    V8[:, b, bass.ts(tci, 4), :], ps[0][:]
                                )

                        units.append(u)
                return units

            pending = {}       # key -> deque of unit callables
            order = deque()    # key pop order
            drain_q = deque()
            late_q = deque()

            def push(key, units):
                pending[key] = deque(units)
                order.append(key)

            def flush(key):
                q = pending.get(key)
                while q:
                    q.popleft()()

            def pop_fill(n):
                for _ in range(n):
                    if drain_q:
                        drain_q.popleft()()
                        continue
                    while order and not pending.get(order[0]):
                        order.popleft()
                    if order:
                        pending[order[0]].popleft()()

            prev = {}

            def make_drain(b, q_i, PT, ps_o, ps_r):
                """AV + rowsum of chunk (b, q_i): 4 units x (2+2) DR MMs."""
                units = []
                for g in range(4):

                    def uav(g=g, b=b, PT=PT, ps_o=ps_o, ps_r=ps_r):
                        for kp in (2 * g, 2 * g + 1):
                            nc.tensor.matmul(
                                ps_o[:],
                                V8[:, b, bass.ts(kp, 2), :],
                                PT[:, bass.ts(kp, 2), :],
                                start=(kp == 0), stop=(kp == KT_N // 2 - 1),
                                perf_mode=PM_DR, skip_group_check=True,
                            )
                            nc.tensor.matmul(
                                ps_r[:],
                                ones8[:],
                                PT[:, bass.ts(kp, 2), :],
                                start=(kp == 0), stop=(kp == KT_N // 2 - 1),
                                perf_mode=PM_DR, skip_group_check=True,
                            )

                    units.append(uav)
                return units

            def make_fin(b, q_i, ps_o, ps_r):
                """1/r + attention normalize + residual add + bn_stats."""

                def ufin(b=b, q_i=q_i, ps_o=ps_o, ps_r=ps_r):
                    rb = rbp.tile([P, TCW], F32, tag="rb")
                    nc.vector.reciprocal_approx_fast(out=rb[:], in_=ps_r[:])
                    otmp = otmpp.tile([P, TCW], F32, tag="ot")
                    nc.vector.tensor_tensor(otmp[:], ps_o[:], rb[:], ALU.mult)
                    dst = o_res[:, b, bass.ts(q_i, TCW)]
                    nc.vector.tensor_add(dst, dst, otmp[:])
                    nc.vector.bn_stats(out=bstat[:, b * TC + q_i, :], in_=dst)

                return ufin

            def emit_chunk(b, q_i):
                # correctness fences: everything this chunk's matmuls read
                # must already be emitted (program order defines deps)
                flush(("k", b))
                flush(("q", b, q_i))
                PT = ptp.tile([P, KT_N, TCW], FP8, tag="pt")
                if prev:
                    flush(("v", prev["b"]))   # AV drain needs V tiles
                    drain_q.extend(
                        make_drain(prev["b"], prev["q_i"], prev["PT"],
                                   prev["ps_o"], prev["ps_r"])
                    )
                    late_q.append(
                        make_fin(prev["b"], prev["q_i"], prev["ps_o"],
                                 prev["ps_r"])
                    )
                ps_o = opsum.tile([P, TCW], F32, tag="op", name="ps_o")
                ps_r = opsum.tile([P, TCW], F32, tag="op", name="ps_r")
                for kp in range(KT_N // 2):
                    ps_s = spsum.tile([P, 2, TCW], F32, tag="sp")
                    for j in range(2):
                        nc.tensor.matmul(
                            ps_s[:, j, :],
                            KT[:, b, 2 * kp + j, :],
                            QT[:, b, bass.ts(q_i, TCW)],
                            start=True, stop=True, skip_group_check=True,
                        )
                    nc.scalar.activation(
                        out=PT[:, bass.ts(kp, 2), :], in_=ps_s[:], func=AF.Exp
                    )
                    if kp == 6 and late_q:
                        late_q.popleft()()
                        pop_fill(1)
                    else:
                        pop_fill(2)
                prev.clear()
                prev.update({"b": b, "q_i": q_i, "PT": PT, "ps_o": ps_o,
                             "ps_r": ps_r})

            # ---- emission ----
            ku0 = kproj_units(0)
            qu0 = [qproj_units(0, tci) for tci in range(TC)]
            for tci in range(TC):
                for u in qu0[tci]:
                    u()
                for u in ku0[2 * tci : 2 * tci + 2]:
                    u()

            for b in range(B):
                for q_i in range(TC):
                    if b == 0 and q_i == 0:
                        push(("v", 0), vproj_units(0))
                    if q_i == 0 and b < B - 1:
                        dma_batch(b + 1)
                    if q_i == 1 and b < B - 1:
                        push(("k", b + 1), kproj_units(b + 1))
                    if q_i == 2 and b < B - 1:
                        push(("q", b + 1, 0), qproj_units(b + 1, 0))
                        push(("v", b + 1), vproj_units(b + 1))
                    if q_i < TC - 1 and b > 0:
                        push(("q", b, q_i + 1), qproj_units(b, q_i + 1))
                    emit_chunk(b, q_i)

            # drain the last chunk + any remaining stragglers
            flush(("v", prev["b"]))
            drain_q.extend(
                make_drain(prev["b"], prev["q_i"], prev["PT"], prev["ps_o"],
                           prev["ps_r"])
            )
            late_q.append(
                make_fin(prev["b"], prev["q_i"], prev["ps_o"], prev["ps_r"])
            )
            nc.scalar.activation(out=warm[:], in_=eps_t[:], func=AF.Sqrt)
            while drain_q or any(pending.get(k) for k in list(order)):
                pop_fill(1)
            while late_q:
                late_q.popleft()()

            # ---- batchnorm finale (fully local) ----
            nc.vector.bn_aggr(out=mv[:], in_=bstat[:])
            nc.scalar.activation(
                out=stdt[:], in_=mv[:, 1:2], func=AF.Sqrt, bias=eps_t[:]
            )
            nc.vector.reciprocal(out=rstd[:], in_=stdt[:])
            nc.vector.tensor_mul(scl[:], gam[:], rstd[:])
            nc.vector.tensor_mul(shf[:], mv[:, 0:1], scl[:])
            nc.vector.tensor_sub(shf[:], bet[:], shf[:])
            for b in range(B):
                for half in range(2):
                    sl = bass.ts(half, S // 2)
                    src = o_res[:, b, sl]
                    if half == 0:
                        nc.vector.tensor_scalar(
                            src, src, scl[:], shf[:], ALU.mult, ALU.add
                        )
                    else:
                        nc.scalar.activation(
                            out=src, in_=src, func=AF.Identity,
                            bias=shf[:], scale=scl[:],
                        )
                weng = nc.sync if b % 2 == 0 else nc.scalar
                weng.dma_start(out[:, b, :], o_res[:, b, :])

    nc.finalize()
    return nc


_NC = None


def _get_nc():
    global _NC
    if _NC is None:
        _NC = _build()
    return _NC


def _make_in_maps(query, keys, Wq, Wk, Wv, gamma, beta):
    query = np.asarray(query, dtype=np.float32)
    keys = np.asarray(keys, dtype=np.float32)
    Wq = np.asarray(Wq, dtype=np.float32)
    Wk = np.asarray(Wk, dtype=np.float32)
    Wv = np.asarray(Wv, dtype=np.float32)
    gamma = np.asarray(gamma, dtype=np.float32)
    beta = np.asarray(beta, dtype=np.float32)

    qT = np.ascontiguousarray(query.transpose(0, 2, 1))   # (B, D, S)
    kT = np.ascontiguousarray(keys.transpose(0, 2, 1))

    # (B, D, S) -> [B, P, TC, DP, 2, TCW] fp8 (P outermost per batch)
    v = qT.reshape(B, DP, 2, P, TC, TCW).transpose(0, 3, 4, 1, 2, 5)
    qt8 = np.ascontiguousarray(v.astype(NPFP8))

    # (B, D, S) -> [B, P, TC, DT, TCW] bf16
    v = kT.reshape(B, DT, P, TC, TCW).transpose(0, 2, 3, 1, 4)
    kt16 = np.ascontiguousarray(v.astype(NPBF16))

    in_maps = []
    for c in range(8):
        rows = slice(DH * c, DH * (c + 1))

        def packw8(w):  # rows of W -> [P, DP, 2, DH] fp8, scaled x32
            wt = np.ascontiguousarray(w[rows].T * WSCALE)   # (D, 128)
            v = wt.reshape(DP, 2, P, DH).transpose(2, 0, 1, 3)
            return np.ascontiguousarray(v.astype(NPFP8))

        wv_t = np.ascontiguousarray(Wv[rows].T)             # (D, 128)
        in_maps.append(
            {
                "qt8": qt8,
                "kt16": kt16,
                "wq8": packw8(Wq),
                "wk8": packw8(Wk),
                "wv16": np.ascontiguousarray(
                    wv_t.reshape(DT, P, DH).transpose(1, 0, 2).astype(NPBF16)
                ),
                "qres": np.ascontiguousarray(
                    query[:, :, rows].transpose(0, 2, 1)
                ),  # (B, 128, S)
                "gamma": np.ascontiguousarray(gamma[rows].reshape(P, 1)),
                "beta": np.ascontiguousarray(beta[rows].reshape(P, 1)),
            }
        )
    return in_maps


def _run(in_maps, trace=False, **kw):
    nc = _get_nc()
    return run_bass_kernel_spmd(
        nc, in_maps, core_ids=list(range(8)), trace=trace, **kw
    )


def kernel(query, keys, Wq, Wk, Wv, gamma, beta):
    in_maps = _make_in_maps(query, keys, Wq, Wk, Wv, gamma, beta)
    res = _run(in_maps)
    output = np.empty((B, S, D), dtype=np.float32)
    for c in range(8):
        oc = res.results[c]["out"]                    # (128, B, S)
        output[:, :, DH * c : DH * (c + 1)] = oc.transpose(1, 2, 0)
    return output


# revision 11
# speedup vs baseline: 1.2427x; 1.1642x over previous
"""Multi-head attention + residual + batchnorm on 8 trn2 NeuronCores.

Sharding: core c owns head h = c for ALL 4 batches. Head h covers output
features [h*128, (h+1)*128), so batchnorm statistics over (batch, seq)
are fully local to the core: no cross-core collective at all.

All device compute is feature-major so every matmul contracts over the
partition dim with zero on-chip transposes:

  QT[u,t] = Wq_h @ query[b].T     fp8 DoubleRow (K=256/pass), descaled
  KT[u,t] = Wk_h @ keys[b].T      fp8 DoubleRow from on-chip-cast keys
  V[t,u]  = keys[b] @ Wv_h.T      bf16 (N=128 matmuls, FWL), stored fp8
  ST[k,q] = KT.T-contract QT      bf16 (K=128: DoubleRow not applicable)
  PT      = exp(ST)               ACT, PSUM->SBUF, fp8 (scores bounded)
  OT[u,q] = sum_k V[k,u]*PT[k,q]  fp8 DoubleRow
  r[q]    = sum_k PT[k,q]         fp8 DoubleRow ones-matmuls (f32 acc)
  o_res   = OT/r + query[b].T     f32 residual
  batchnorm over (b,s): bn_stats per chunk, local bn_aggr, affine.

fp8 scaling: weights are scaled x32 on host (unit std, fits e4m3);
the 1/32 score scale plus the x32x32 weight descale is folded into the
QT/KT PSUM copy-out factor sq = sk = 1/sqrt(32768).

DMA strategy (per-queue throughput is descriptor-rate-bound, ~23
descriptors/us): keys ship once as bf16 (the K-projection's fp8 copy
is cast on-chip by the DVE), and the host layouts put [P] outermost so
every DMA moves 4-16KB per partition in one descriptor. Streams are
spread over the gpsimd (keys), sync (query-fp8), and scalar (residual
+ weights) queues.

The tensor-engine emission interleaves, between the score matmuls of
chunk N, the AV+rowsum drain of chunk N-1 plus projection work for
upcoming batches, so the PE never stalls on the ACT engine (exp is the
per-chunk ACT straggler) freeing score PSUM banks, and the HAM clock
gate stays at 8/8.
"""
import sys

sys.path.insert(0, "/opt/trn_rl_repo")

from collections import deque

import numpy as np

import concourse.bass as bass
import concourse.tile as tile
from concourse import bacc, mybir
from concourse.bass_utils import run_bass_kernel_spmd

F32 = mybir.dt.float32
BF16 = mybir.dt.bfloat16
FP8 = mybir.dt.float8e4
AF = mybir.ActivationFunctionType
PM_DR = mybir.MatmulPerfMode.DoubleRow
ALU = mybir.AluOpType
NPBF16 = mybir.dt.np(BF16)
NPFP8 = mybir.dt.np(FP8)

B, S, D, H = 4, 2048, 1024, 8
DH = 128
P = 128
TC = 4                # 512-token chunks per sequence
TCW = 512
DT = 8                # 128-wide d-tiles in D
DP = 4                # d-tile pairs (DoubleRow K=256)
KT_N = 16             # 128-wide k-tiles per sequence
EPS = 1e-5
WSCALE = 32.0
SQK = 1.0 / np.sqrt(32768.0)   # QT/KT copy-out descale; sq*sk*D = 1/32


def _build():
    nc = bacc.Bacc(num_swdge_queues=1)
    qt8 = nc.declare_dram_parameter(
        "qt8", [B, P, TC, DP, 2, TCW], FP8, isOutput=False)
    kt16 = nc.declare_dram_parameter(
        "kt16", [B, P, TC, DT, TCW], BF16, isOutput=False)
    wq8 = nc.declare_dram_parameter("wq8", [P, DP, 2, DH], FP8, isOutput=False)
    wk8 = nc.declare_dram_parameter("wk8", [P, DP, 2, DH], FP8, isOutput=False)
    wv16 = nc.declare_dram_parameter("wv16", [P, DT, DH], BF16, isOutput=False)
    qres = nc.declare_dram_parameter("qres", [B, P, S], F32, isOutput=False)
    gamma = nc.declare_dram_parameter("gamma", [P, 1], F32, isOutput=False)
    beta = nc.declare_dram_parameter("beta", [P, 1], F32, isOutput=False)
    out = nc.declare_dram_parameter("out", [P, B, S], F32, isOutput=True)

    with tile.TileContext(nc) as tc:
        with (
            tc.tile_pool(name="persist", bufs=1) as persist,
            tc.tile_pool(name="xq8", bufs=2) as xq8p,       # per-batch tiles
            tc.tile_pool(name="xk16", bufs=6) as xk16p,     # per-tc tiles
            tc.tile_pool(name="kf8", bufs=6) as kf8p,       # cast scratch
            tc.tile_pool(name="pt", bufs=2) as ptp,
            tc.tile_pool(name="rb", bufs=2) as rbp,
            tc.tile_pool(name="otmp", bufs=2) as otmpp,
            tc.tile_pool(name="ppsum", bufs=2, space="PSUM") as ppsum,
            tc.tile_pool(name="spsum", bufs=2, space="PSUM") as spsum,
            tc.tile_pool(name="opsum", bufs=2, space="PSUM") as opsum,
        ):
            # ---- persistent SBUF ----
            QT = persist.tile([P, B, S], BF16)            # (dh, b, q) 16KB/p
            KT = persist.tile([P, B, KT_N, P], BF16)      # (dh, b, kt, k) 16KB/p
            V8 = persist.tile([P, B, KT_N, DH], FP8)      # (t128, b, kt, u) 8KB/p
            o_res = persist.tile([P, B, S], F32)          # 32KB/p
            bstat = persist.tile([P, B * TC, nc.vector.BN_STATS_DIM], F32)
            wq_s = persist.tile([P, DP, 2, DH], FP8)
            wk_s = persist.tile([P, DP, 2, DH], FP8)
            wv_s = persist.tile([P, DT, DH], BF16)
            gam = persist.tile([P, 1], F32)
            bet = persist.tile([P, 1], F32)
            ones_b = persist.tile([P, P], BF16)
            ones8 = persist.tile([P, 2, P], FP8)
            eps_t = persist.tile([P, 1], F32)
            warm = persist.tile([P, 1], F32)
            mv = persist.tile([P, 2], F32)
            stdt = persist.tile([P, 1], F32)
            rstd = persist.tile([P, 1], F32)
            scl = persist.tile([P, 1], F32)
            shf = persist.tile([P, 1], F32)

            # ---- preamble ----
            nc.vector.memset(eps_t[:], float(EPS))
            nc.vector.memset(ones_b[:], 1.0)
            nc.vector.memset(ones8[:], 1.0)

            xtiles = {}   # streamed tiles keyed by (kind, b[, tc])

            # batch-0 keys stream first (K-proj gates everything),
            # spread across all three DMA queues
            qeng = [nc.gpsimd, nc.sync, nc.scalar, nc.gpsimd]
            nc.scalar.dma_start(wk_s[:], wk8[:])
            for tci in range(TC):
                t = xk16p.tile([P, DT, TCW], BF16, tag="xk16", name="t")
                qeng[tci].dma_start(t[:], kt16[0, :, tci])
                xtiles[("k16", 0, tci)] = t
            nc.scalar.dma_start(wq_s[:], wq8[:])
            t0q = xq8p.tile([P, TC, DP, 2, TCW], FP8, tag="xq", name="t0q")
            for tci in range(TC):
                nc.sync.dma_start(t0q[:, tci], qt8[0, :, tci])
            xtiles[("q8", 0)] = t0q
            nc.scalar.dma_start(wv_s[:], wv16[:])
            nc.sync.dma_start(o_res[:, 0, :], qres[0])
            nc.scalar.dma_start(gam[:], gamma[:])
            nc.scalar.dma_start(bet[:], beta[:])
            nc.scalar.activation(out=warm[:], in_=eps_t[:], func=AF.Exp)

            # PE warmup: pull the HAM clock gate to 8/8 before real work
            wps = ppsum.tile([P, TCW], F32, tag="pp", name="wps")
            for _ in range(20):
                nc.tensor.matmul(
                    wps[:, 0:P], ones_b[:], ones_b[:],
                    start=True, stop=True, skip_group_check=True,
                )

            def dma_batch(b):
                for tci in range(TC):
                    t = xk16p.tile([P, DT, TCW], BF16, tag="xk16")
                    qeng[(b + tci) % 3].dma_start(t[:], kt16[b, :, tci])
                    xtiles[("k16", b, tci)] = t
                t = xq8p.tile([P, TC, DP, 2, TCW], FP8, tag="xq")
                nc.sync.dma_start(t[:], qt8[b])
                xtiles[("q8", b)] = t
                nc.scalar.dma_start(o_res[:, b, :], qres[b])

            # ---- tensor-work units (each ~0.3-0.7us of PE time) ----
            def kproj_units(b):
                units = []
                for tci in range(TC):
                    ps = [None]

                    def u1(b=b, tci=tci, ps=ps):
                        ps[0] = ppsum.tile([P, TC, P], F32, tag="pp", name="pk")
                        xt = xtiles[("k16", b, tci)]
                        for dp in range(2):
                            kf = kf8p.tile([P, 2, TCW], FP8, tag="kf", name="kf")
                            nc.vector.tensor_copy(
                                kf[:], xt[:, bass.ts(dp, 2), :]
                            )
                            nc.tensor.matmul(
                                ps[0][:], wk_s[:, dp], kf[:],
                                start=(dp == 0), stop=False,
                                perf_mode=PM_DR, skip_group_check=True,
                            )

                    def u2(b=b, tci=tci, ps=ps):
                        xt = xtiles[("k16", b, tci)]
                        for dp in range(2, DP):
                            kf = kf8p.tile([P, 2, TCW], FP8, tag="kf", name="kf")
                            nc.vector.tensor_copy(
                                kf[:], xt[:, bass.ts(dp, 2), :]
                            )
                            nc.tensor.matmul(
                                ps[0][:], wk_s[:, dp], kf[:],
                                start=False, stop=(dp == DP - 1),
                                perf_mode=PM_DR, skip_group_check=True,
                            )
                        nc.vector.tensor_scalar(
                            KT[:, b, bass.ts(tci, TC), :], ps[0][:],
                            float(SQK), None, ALU.mult,
                        )

                    units += [u1, u2]
                return units

            def qproj_units(b, tci):
                ps = [None]

                def u1(b=b, tci=tci, ps=ps):
                    ps[0] = ppsum.tile([P, TCW], F32, tag="pp", name="pq")
                    xt = xtiles[("q8", b)]
                    for dp in range(2):
                        nc.tensor.matmul(
                            ps[0][:], wq_s[:, dp], xt[:, tci, dp],
                            start=(dp == 0), stop=False,
                            perf_mode=PM_DR, skip_group_check=True,
                        )

                def u2(b=b, tci=tci, ps=ps):
                    xt = xtiles[("q8", b)]
                    for dp in range(2, DP):
                        nc.tensor.matmul(
                            ps[0][:], wq_s[:, dp], xt[:, tci, dp],
                            start=False, stop=(dp == DP - 1),
                            perf_mode=PM_DR, skip_group_check=True,
                        )
                    nc.vector.tensor_scalar(
                        QT[:, b, bass.ts(tci, TCW)], ps[0][:],
                        float(SQK), None, ALU.mult,
                    )

                return [u1, u2]

            def vproj_units(b):
                units = []
                for tci in range(TC):
                    ps = [None]
                    for sub in range(4):

                        def u(b=b, tci=tci, sub=sub, ps=ps):
                            if sub == 0:
                                ps[0] = ppsum.tile([P, 4, DH], F32, tag="pp",
                                                   name="pv")
                            xt = xtiles[("k16", b, tci)]
                            for d in range(DT):
                                nc.tensor.matmul(
                                    ps[0][:, sub, :],
                                    xt[:, d, bass.ts(sub, P)],
                                    wv_s[:, d, :],
                                    start=(d == 0), stop=(d == DT - 1),
                                    skip_group_check=True,
                                )
                            if sub == 3:
                                nc.vector.tensor_copy(
                                    V8[:, b, bass.ts(tci, 4), :], ps[0][:]
                                )

                        units.append(u)
                return units

            pending = {}       # key -> deque of unit callables
            order = deque()    # key pop order
            drain_q = deque()
            late_q = deque()

            def push(key, units):
                pending[key] = deque(units)
                order.append(key)

            def flush(key):
                q = pending.get(key)
                while q:
                    q.popleft()()

            def pop_fill(n):
                for _ in range(n):
                    if drain_q:
                        drain_q.popleft()()
                        continue
                    while order and not pending.get(order[0]):
                        order.popleft()
                    if order:
                        pending[order[0]].popleft()()

            prev = {}

            def make_drain(b, q_i, PT, ps_o, ps_r):
                """AV + rowsum of chunk (b, q_i): 4 units x (2+2) DR MMs."""
                units = []
                for g in range(4):

                    def uav(g=g, b=b, PT=PT, ps_o=ps_o, ps_r=ps_r):
                        for kp in (2 * g, 2 * g + 1):
                            nc.tensor.matmul(
                                ps_o[:],
                                V8[:, b, bass.ts(kp, 2), :],
                                PT[:, bass.ts(kp, 2), :],
                                start=(kp == 0), stop=(kp == KT_N // 2 - 1),
                                perf_mode=PM_DR, skip_group_check=True,
                            )
                            nc.tensor.matmul(
                                ps_r[:],
                                ones8[:],
                                PT[:, bass.ts(kp, 2), :],
                                start=(kp == 0), stop=(kp == KT_N // 2 - 1),
                                perf_mode=PM_DR, skip_group_check=True,
                            )

                    units.append(uav)
                return units

            def make_fin(b, q_i, ps_o, ps_r):
                """1/r + attention normalize + residual add + bn_stats."""

                def ufin(b=b, q_i=q_i, ps_o=ps_o, ps_r=ps_r):
                    rb = rbp.tile([P, TCW], F32, tag="rb")
                    nc.vector.reciprocal_approx_fast(out=rb[:], in_=ps_r[:])
                    otmp = otmpp.tile([P, TCW], F32, tag="ot")
                    nc.vector.tensor_tensor(otmp[:], ps_o[:], rb[:], ALU.mult)
                    dst = o_res[:, b, bass.ts(q_i, TCW)]
                    nc.vector.tensor_add(dst, dst, otmp[:])
                    nc.vector.bn_stats(out=bstat[:, b * TC + q_i, :], in_=dst)

                return ufin

            def emit_chunk(b, q_i):
                # correctness fences: everything this chunk's matmuls read
                # must already be emitted (program order defines deps)
                flush(("k", b))
                flush(("q", b, q_i))
                PT = ptp.tile([P, KT_N, TCW], FP8, tag="pt")
                if prev:
                    flush(("v", prev["b"]))   # AV drain needs V tiles
                    drain_q.extend(
                        make_drain(prev["b"], prev["q_i"], prev["PT"],
                                   prev["ps_o"], prev["ps_r"])
                    )
                    late_q.append(
                        make_fin(prev["b"], prev["q_i"], prev["ps_o"],
                                 prev["ps_r"])
                    )
                ps_o = opsum.tile([P, TCW], F32, tag="op", name="ps_o")
                ps_r = opsum.tile([P, TCW], F32, tag="op", name="ps_r")
                for kp in range(KT_N // 2):
                    ps_s = spsum.tile([P, 2, TCW], F32, tag="sp")
                    for j in range(2):
                        nc.tensor.matmul(
                            ps_s[:, j, :],
                            KT[:, b, 2 * kp + j, :],
                            QT[:, b, bass.ts(q_i, TCW)],
                            start=True, stop=True, skip_group_check=True,
                        )
                    nc.scalar.activation(
                        out=PT[:, bass.ts(kp, 2), :], in_=ps_s[:], func=AF.Exp
                    )
                    if kp == 6 and late_q:
                        late_q.popleft()()
                        pop_fill(1)
                    else:
                        pop_fill(2)
                prev.clear()
                prev.update({"b": b, "q_i": q_i, "PT": PT, "ps_o": ps_o,
                             "ps_r": ps_r})

            # ---- emission ----
            for u in kproj_units(0):
                u()
            for u in qproj_units(0, 0):
                u()

            for b in range(B):
                for q_i in range(TC):
                    if b == 0 and q_i == 0:
                        push(("v", 0), vproj_units(0))
                    if q_i == 0 and b < B - 1:
                        dma_batch(b + 1)
                    if q_i == 1 and b < B - 1:
                        push(("k", b + 1), kproj_units(b + 1))
                    if q_i == 2 and b < B - 1:
                        push(("q", b + 1, 0), qproj_units(b + 1, 0))
                        push(("v", b + 1), vproj_units(b + 1))
                    if q_i < TC - 1:
                        push(("q", b, q_i + 1), qproj_units(b, q_i + 1))
                    emit_chunk(b, q_i)

            # drain the last chunk + any remaining stragglers
            flush(("v", prev["b"]))
            drain_q.extend(
                make_drain(prev["b"], prev["q_i"], prev["PT"], prev["ps_o"],
                           prev["ps_r"])
            )
            late_q.append(
                make_fin(prev["b"], prev["q_i"], prev["ps_o"], prev["ps_r"])
            )
            nc.scalar.activation(out=warm[:], in_=eps_t[:], func=AF.Sqrt)
            while drain_q or any(pending.get(k) for k in list(order)):
                pop_fill(1)
            while late_q:
                late_q.popleft()()

            # ---- batchnorm finale (fully local) ----
            nc.vector.bn_aggr(out=mv[:], in_=bstat[:])
            nc.scalar.activation(
                out=stdt[:], in_=mv[:, 1:2], func=AF.Sqrt, bias=eps_t[:]
            )
            nc.vector.reciprocal(out=rstd[:], in_=stdt[:])
            nc.vector.tensor_mul(scl[:], gam[:], rstd[:])
            nc.vector.tensor_mul(shf[:], mv[:, 0:1], scl[:])
            nc.vector.tensor_sub(shf[:], bet[:], shf[:])
            for b in range(B):
                for half in range(2):
                    sl = bass.ts(half, S // 2)
                    src = o_res[:, b, sl]
                    if half == 0:
                        nc.vector.tensor_scalar(
                            src, src, scl[:], shf[:], ALU.mult, ALU.add
                        )
                    else:
                        nc.scalar.activation(
                            out=src, in_=src, func=AF.Identity,
                            bias=shf[:], scale=scl[:],
                        )
                weng = nc.sync if b % 2 == 0 else nc.scalar
                weng.dma_start(out[:, b, :], o_res[:, b, :])

    nc.finalize()
    return nc


_NC = None


def _get_nc():
    global _NC
    if _NC is None:
        _NC = _build()
    return _NC


def _make_in_maps(query, keys, Wq, Wk, Wv, gamma, beta):
    query = np.asarray(query, dtype=np.float32)
    keys = np.asarray(keys, dtype=np.float32)
    Wq = np.asarray(Wq, dtype=np.float32)
    Wk = np.asarray(Wk, dtype=np.float32)
    Wv = np.asarray(Wv, dtype=np.float32)
    gamma = np.asarray(gamma, dtype=np.float32)
    beta = np.asarray(beta, dtype=np.float32)

    qT = np.ascontiguousarray(query.transpose(0, 2, 1))   # (B, D, S)
    kT = np.ascontiguousarray(keys.transpose(0, 2, 1))

    # (B, D, S) -> [B, P, TC, DP, 2, TCW] fp8 (P outermost per batch)
    v = qT.reshape(B, DP, 2, P, TC, TCW).transpose(0, 3, 4, 1, 2, 5)
    qt8 = np.ascontiguousarray(v.astype(NPFP8))

    # (B, D, S) -> [B, P, TC, DT, TCW] bf16
    v = kT.reshape(B, DT, P, TC, TCW).transpose(0, 2, 3, 1, 4)
    kt16 = np.ascontiguousarray(v.astype(NPBF16))

    in_maps = []
    for c in range(8):
        rows = slice(DH * c, DH * (c + 1))

        def packw8(w):  # rows of W -> [P, DP, 2, DH] fp8, scaled x32
            wt = np.ascontiguousarray(w[rows].T * WSCALE)   # (D, 128)
            v = wt.reshape(DP, 2, P, DH).transpose(2, 0, 1, 3)
            return np.ascontiguousarray(v.astype(NPFP8))

        wv_t = np.ascontiguousarray(Wv[rows].T)             # (D, 128)
        in_maps.append(
            {
                "qt8": qt8,
                "kt16": kt16,
                "wq8": packw8(Wq),
                "wk8": packw8(Wk),
                "wv16": np.ascontiguousarray(
                    wv_t.reshape(DT, P, DH).transpose(1, 0, 2).astype(NPBF16)
                ),
                "qres": np.ascontiguousarray(
                    query[:, :, rows].transpose(0, 2, 1)
                ),  # (B, 128, S)
                "gamma": np.ascontiguousarray(gamma[rows].reshape(P, 1)),
                "beta": np.ascontiguousarray(beta[rows].reshape(P, 1)),
            }
        )
    return in_maps


def _run(in_maps, trace=False, **kw):
    nc = _get_nc()
    return run_bass_kernel_spmd(
        nc, in_maps, core_ids=list(range(8)), trace=trace, **kw
    )


def kernel(query, keys, Wq, Wk, Wv, gamma, beta):
    in_maps = _make_in_maps(query, keys, Wq, Wk, Wv, gamma, beta)
    res = _run(in_maps)
    output = np.empty((B, S, D), dtype=np.float32)
    for c in range(8):
        oc = res.results[c]["out"]                    # (128, B, S)
        output[:, :, DH * c : DH * (c + 1)] = oc.transpose(1, 2, 0)
    return output
